# revision 33
# baseline (speedup 1.0000x reference)
"""Bass/Tile TRN2 kernel for nn_AverageAttention (cumavg -> LN -> FFN -> sigmoid gating).

Sharding: data-parallel over batch, one batch element per NeuronCore (B=8, 8 cores).

Per-core pipeline (L=2048 tokens in 4 quarters of 512 = 4 tiles of 128):
  phase A (per 128-token tile, natural [t, d] layout):
     cumavg via triu-matmul + running-prefix (strict-lower-tril matmul) in a
     persistent PSUM region; avg scale split ACT/DVE; LayerNorm stats via
     bn_stats/bn_aggr with rstd computed on DVE via fast-inverse-sqrt
     (bit hack + 1 Newton step, batched per tile pair) so the ACT engine
     never loads the sqrt table -- the only table function left is Sigmoid
     (one ACT_TABLE_LOAD for the whole kernel instead of ~21).
     PE-transposes produce avgT (f16) / x-chunks of catq8 (fp8) in [d, t]
     layout; lnT transposes are batched per pair after the normalize.
  phase B (per quarter): y1T = w1'@lnT (relu+b1 on ACT, fp16), y2T = w2@r1T,
     ffnT = y2T + b2 + avgT (f16); ffnT cast into catq8 (fp8) on GPSIMD;
     ffnT transposed back to natural layout (fnat, f16) and DMA'd out per
     128-token row block (scalar queue).
  phase C (per 128-token tile, natural output layout): gating computed as
     gate[t, j] = sum_c catT8[c-pair]^T @ gw8[c-pair, j] with fp8 DoubleRow
     matmuls. Stationary = catq8 chunk-pair x t-tile, moving = gw8
     (SBUF-resident). sigmoid -> f16; the two sig*x / sig*ffn products run
     on the otherwise-idle GPSIMD engine (DVE for the last tile to shorten
     the tail), final add on DVE, out stored f16 on the vector queue.
     C tiles 0-1 of each quarter are emitted between the two FFN halves so
     their epilogues overlap B1's matmuls.

Weights stream on three DMA queues (scalar/vector/gpsimd) with w1 first so
phase B(0) is not gated on a single queue; consts+x ride sync. Outputs are
f16 (upcast to f32 on the host). ln_g/ln_b folded into w1/b1 on host.
"""

import numpy as np

B, L, D = 8, 2048, 1024
P = 128
NT = L // P          # 16 token tiles
KC = D // P          # 8 d-chunks
QT = 4               # tiles per quarter
NQ = NT // QT        # 4 quarters
QW = QT * P          # 512 tokens per quarter
EPS = 1e-6

_CACHE = {}


def _build(has_bias):
    key = ("nc", has_bias)
    if key in _CACHE:
        return _CACHE[key]

    import concourse.bacc as bacc
    import concourse.mybir as mybir
    import concourse.tile as tile
    from contextlib import ExitStack

    f32 = mybir.dt.float32
    f32r = mybir.dt.float32r
    f16 = mybir.dt.float16
    f8 = mybir.dt.float8e4
    i32 = mybir.dt.int32
    Alu = mybir.AluOpType
    Act = mybir.ActivationFunctionType
    DR = mybir.MatmulPerfMode.DoubleRow

    GCH = 18 if has_bias else 16   # gating contraction chunks
    NCP = GCH // 2                 # chunk pairs per gate psum fill

    nc = bacc.Bacc("TRN2", debug=False, target_bir_lowering=False, num_devices=B)

    x_d = nc.dram_tensor("x", [L, D], f16, kind="ExternalInput").ap()
    xt8_d = nc.dram_tensor("xt8", [NQ, KC, P, QW], f8, kind="ExternalInput").ap()
    w1_d = nc.dram_tensor("w1", [KC, P, D], f16, kind="ExternalInput").ap()
    b1_d = nc.dram_tensor("b1", [P, KC], f32, kind="ExternalInput").ap()
    w2_d = nc.dram_tensor("w2", [KC, P, D], f16, kind="ExternalInput").ap()
    b2_d = nc.dram_tensor("b2", [P, KC], f32, kind="ExternalInput").ap()
    gw8_d = nc.dram_tensor("gw8", [P, GCH * 2 * D], f8, kind="ExternalInput").ap()
    if has_bias:
        ones2_d = nc.dram_tensor("ones2", [P, 2 * P], f8, kind="ExternalInput").ap()
    inv_d = nc.dram_tensor("invsteps", [P, NT], f32, kind="ExternalInput").ap()
    triu_d = nc.dram_tensor("triu", [P, P], f16, kind="ExternalInput").ap()
    stril_d = nc.dram_tensor("stril", [P, P], f16, kind="ExternalInput").ap()
    ident_d = nc.dram_tensor("ident", [P, P], f32r, kind="ExternalInput").ap()
    ident16_d = nc.dram_tensor("ident16", [P, P], f16, kind="ExternalInput").ap()
    out_d = nc.dram_tensor("out", [L, D], f16, kind="ExternalOutput").ap()
    ffn_d = nc.dram_tensor("ffn", [L, D], f16, kind="ExternalOutput").ap()

    def r(ap):
        return ap.bitcast(f32r)

    def v(ap):
        return ap.bitcast(f32)

    def wide3(ap, inner=P):
        return ap.rearrange("p (b t) -> p b t", t=inner)

    with tile.TileContext(nc) as tc, ExitStack() as ctx:
        consts = ctx.enter_context(tc.tile_pool(name="consts", bufs=1))
        wts = ctx.enter_context(tc.tile_pool(name="wts", bufs=1))
        quartA = ctx.enter_context(tc.tile_pool(name="quartA", bufs=2))
        quartB = ctx.enter_context(tc.tile_pool(name="quartB", bufs=1))
        xload = ctx.enter_context(tc.tile_pool(name="xload", bufs=8))
        avgp = ctx.enter_context(tc.tile_pool(name="avgp", bufs=2))
        statp = ctx.enter_context(tc.tile_pool(name="statp", bufs=2))
        sigp = ctx.enter_context(tc.tile_pool(name="sigp", bufs=3))
        tmpp = ctx.enter_context(tc.tile_pool(name="tmpp", bufs=2))
        psA_p = ctx.enter_context(tc.tile_pool(name="psA", bufs=1, space="PSUM"))
        ps512 = ctx.enter_context(tc.tile_pool(name="ps512", bufs=2, space="PSUM"))
        gate_p = ctx.enter_context(tc.tile_pool(name="gate", bufs=2, space="PSUM"))

        # startup DMA layout. Only sync (SP) and scalar (ACT) rings have
        # hardware DGE -- gpsimd DMAs go through a slow software path, so the
        # gpsimd ring carries ONLY x0 + small consts (needed in the first
        # ~10us, tiny). All weights ride the two HW rings: w1 split across
        # both (phase B(0) needs it ~15us in), then w2/gw8. Small consts
        # never go ahead of x on a HW ring -- their 128-row tiny descriptors
        # starve the x loads for ~25us.
        xi_pre = [xload.tile([P, D], f16, name=f"xi_{i}", tag="xi")
                  for i in range(QT)]
        nc.gpsimd.dma_start(out=xi_pre[0], in_=x_d[0:P, :])
        triu = consts.tile([P, P], f16, name="triu_sb")
        nc.scalar.dma_start(out=triu, in_=triu_d)
        stril = consts.tile([P, P], f16, name="stril_sb")
        nc.scalar.dma_start(out=stril, in_=stril_d)
        ident16 = consts.tile([P, P], f16, name="ident16_sb")
        nc.scalar.dma_start(out=ident16, in_=ident16_d)
        for i in range(1, QT):
            nc.sync.dma_start(out=xi_pre[i], in_=x_d[i * P:(i + 1) * P, :])
        ident = consts.tile([P, P], f32r, name="ident_sb")
        nc.gpsimd.dma_start(out=ident, in_=ident_d)
        inv_sb = consts.tile([P, NT], f32, name="inv_sb")
        nc.gpsimd.dma_start(out=inv_sb, in_=inv_d)
        b1_sb = consts.tile([P, KC], f32, name="b1_sb")
        nc.gpsimd.dma_start(out=b1_sb, in_=b1_d)
        b2_sb = consts.tile([P, KC], f32, name="b2_sb")
        nc.gpsimd.dma_start(out=b2_sb, in_=b2_d)
        if has_bias:
            ones2 = consts.tile([P, 2 * P], f8, name="ones2_sb")
            nc.gpsimd.dma_start(out=ones2, in_=ones2_d)
            o2v = ones2.rearrange("p (s t) -> p s t", s=2)

        # DMA progress is shared across everything queued (descriptor-level
        # fair share), so w2/gw8 dma_starts are DEFERRED into the quarter-0
        # instruction stream: w1 gets the full pipe first and phase B(0)
        # starts ~15us earlier. Tiles are allocated here; dma_starts later.
        w1_sb = [None] * KC
        w2_sb = [None] * KC
        for k in range(KC):
            t1 = wts.tile([P, D], f16, name=f"w1sb{k}", tag=f"w1_{k}")
            (nc.scalar if k % 2 == 0 else nc.sync).dma_start(out=t1, in_=w1_d[k])
            w1_sb[k] = t1
        for k in range(KC):
            w2_sb[k] = wts.tile([P, D], f16, name=f"w2sb{k}", tag=f"w2_{k}")
        gw8 = wts.tile([P, GCH * 2 * D], f8, name="gw8_sb")
        gw_half = (GCH // 2) * 2 * D
        gwv = gw8.rearrange("p (c j) -> p c j", c=GCH)       # [P, GCH, 2048]

        def emit_w2_dmas():
            for k in range(KC):
                nc.scalar.dma_start(out=w2_sb[k], in_=w2_d[k])

        def emit_gw8_dmas():
            nc.sync.dma_start(out=gw8[:, 0:gw_half], in_=gw8_d[:, 0:gw_half])
            nc.scalar.dma_start(out=gw8[:, gw_half:], in_=gw8_d[:, gw_half:])

        # PE warmup: the HAM clock gate keeps the PE at 1.2 GHz until it sees
        # ~3.4us of sustained matmul activity, and transpose-mode work does
        # not count. Burn dummy matmuls on triu (first tensor to arrive)
        # while waiting for x0 so phase A runs at 2.4 GHz.
        for wu in range(3):
            scr = ps512.tile([P, 512], f32, name=f"warm_{wu}", tag="tr")
            for cc in range(16):
                nc.tensor.matmul(scr[:, (cc % 4) * P:(cc % 4 + 1) * P], triu,
                                 triu, start=(cc < 4), stop=(cc >= 12))

        # persistent PSUM region carrying the running column-sum prefix R
        psA = psA_p.tile([P, D], f32, name="psA_t")

        # x tiles and catx (host-transposed fp8 x) for quarter q are
        # prefetched during quarter q-1 (before its out stores hit the sync
        # queue, avoiding head-of-line blocking)
        xq_cur = xi_pre
        catx_cur = quartA.tile([P, KC * QW], f8, name="catx_0", tag="catx")

        def emit_catx_dmas(qq, tile_):
            cv = tile_.rearrange("p (c t) -> p c t", c=KC)
            for c in range(KC):
                eng = nc.sync if c % 2 == 0 else nc.scalar
                eng.dma_start(out=cv[:, c, :], in_=xt8_d[qq, c])

        for q in range(NQ):
            lnT = quartA.tile([P, KC * QW], f16, name=f"lnT_{q}", tag="lnT")
            avgT = quartA.tile([P, KC * QW], f16, name=f"avgT_{q}", tag="avgT")
            catx8 = catx_cur
            catxv = catx8.rearrange("p (c t) -> p c t", c=KC)
            catf8 = quartA.tile([P, KC * QW], f8, name=f"catf_{q}", tag="catf")
            catfv = catf8.rearrange("p (c t) -> p c t", c=KC)
            r1T = quartB.tile([P, KC * QW], f16, name=f"r1T_{q}", tag="r1T")
            ffnT = quartB.tile([P, KC * QW], f16, name=f"ffnT_{q}", tag="ffnT")
            fnat = quartB.tile([P, QT * D], f16, name=f"fnat_{q}", tag="fnat")
            fv = fnat.rearrange("p (t d) -> p t d", t=QT)
            xi_tiles = []
            avgs = {}
            lns = {}
            avg_pend = [None]

            def emit_prefix(ti):
                """x load + triu cumsum + psA readout (avg scale) for tile ti.

                Emitted as early as possible so the ACT/DVE psA reads overlap
                whatever PE work precedes the strict-tril update."""
                i = q * QT + ti
                xi = xq_cur[ti]
                xi_tiles.append(xi)
                for s in range(2):
                    nc.tensor.matmul(psA[:, s * 512:(s + 1) * 512], triu,
                                     xi[:, s * 512:(s + 1) * 512],
                                     start=(i == 0), stop=False)
                avg_i = avgp.tile([P, D], f32r, name=f"avg_{i}", tag="avg")
                nc.scalar.mul(avg_i[:, 0:512], psA[:, 0:512], inv_sb[:, i:i + 1])
                nc.vector.tensor_scalar_mul(avg_i[:, 512:1024], psA[:, 512:1024],
                                            inv_sb[:, i:i + 1])
                return avg_i

            def flush_avgtr():
                """Deferred avg->avgT transposes of the previous tile; called
                at the top of arest so they fill the PE bubble while ACT/DVE
                read psA out (the role x-transposes played before catx moved
                to a host-side DMA)."""
                if avg_pend[0] is None:
                    return
                ti_, avg_, avgT_ = avg_pend[0]
                avg_pend[0] = None
                for g in range(2):
                    pt = ps512.tile([P, 512], f32, name=f"pta{q}_{ti_}_{g}",
                                    tag="tr")
                    for cc in range(4):
                        c = g * 4 + cc
                        nc.tensor.transpose(r(pt[:, cc * P:(cc + 1) * P]),
                                            avg_[:, c * P:(c + 1) * P], ident)
                    dst = wide3(avgT_, QW)[:, g * 4:(g + 1) * 4,
                                           ti_ * P:(ti_ + 1) * P]
                    nc.scalar.copy(dst, wide3(pt))

            def emit_arest(ti, avg_i):
                i = q * QT + ti
                xi = xi_tiles[ti]
                avgs[ti] = avg_i
                # PE bubble filler while ACT/DVE read psA out
                flush_avgtr()
                # psA += strict-lower-tril(x_i)  (now holds R_{i+1})
                for s in range(2):
                    nc.tensor.matmul(psA[:, s * 512:(s + 1) * 512], stril,
                                     xi[:, s * 512:(s + 1) * 512],
                                     start=False, stop=(i == NT - 1))

                # LN stats on avg_i, then per-tile rstd (fast-inverse-sqrt
                # seed + one Newton step on DVE -- no ACT sqrt table) and the
                # normalize, so each tile's chain hides in its own slack
                st6 = statp.tile([P, 12], f32, name=f"st6_{i}", tag="st6")
                nc.vector.bn_stats(st6[:, 0:6], v(avg_i[:, 0:512]))
                nc.vector.bn_stats(st6[:, 6:12], v(avg_i[:, 512:1024]))
                mv = statp.tile([P, 2], f32, name=f"mv_{i}", tag="mv")
                nc.vector.bn_aggr(mv, st6.rearrange("p (g s) -> p g s", g=2))
                avg_pend[0] = (ti, avg_i, avgT)

                ve = statp.tile([P, 1], f32, name=f"ve_{i}", tag="ve")
                nc.vector.tensor_scalar(ve, mv[:, 1:2], EPS, None, op0=Alu.add)
                sd = statp.tile([P, 1], f32, name=f"sd_{i}", tag="sd")
                nc.vector.tensor_scalar(sd.bitcast(i32), ve.bitcast(i32),
                                        1, 0xFFFFFFFF,
                                        op0=Alu.logical_shift_right,
                                        op1=Alu.bitwise_xor)
                nc.vector.tensor_scalar(sd.bitcast(i32), sd.bitcast(i32),
                                        0x5F3759E0, None, op0=Alu.add)
                hh = statp.tile([P, 1], f32, name=f"hh_{i}", tag="hh")
                nc.vector.tensor_tensor(hh, sd, sd, op=Alu.mult)
                nc.vector.tensor_tensor(hh, hh, ve, op=Alu.mult)
                nc.vector.tensor_scalar(hh, hh, -0.5, 1.5,
                                        op0=Alu.mult, op1=Alu.add)
                nc.vector.tensor_tensor(sd, sd, hh, op=Alu.mult)
                # ln = (avg - mean) * rstd into a separate buffer (avg_i
                # stays raw for the deferred avgT transposes)
                ln_i = avgp.tile([P, D], f32r, name=f"ln_{i}", tag="ln")
                nc.vector.tensor_scalar(ln_i, v(avg_i), mv[:, 0:1], sd,
                                        op0=Alu.subtract, op1=Alu.mult)
                lns[ti] = ln_i

            def emit_lntr_pair(t0, t1):
                """lnT transposes for a normalized tile pair (PE + ACT/DVE)."""
                for idx, ti_ in enumerate((t0, t1)):
                    avg_ = lns[ti_]
                    for g in range(2):
                        pt = ps512.tile([P, 512], f32, name=f"ptl{q}_{ti_}_{g}",
                                        tag="tr")
                        for cc in range(4):
                            c = g * 4 + cc
                            nc.tensor.transpose(r(pt[:, cc * P:(cc + 1) * P]),
                                                avg_[:, c * P:(c + 1) * P], ident)
                        dst = wide3(lnT, QW)[:, g * 4:(g + 1) * 4,
                                             ti_ * P:(ti_ + 1) * P]
                        if (idx + g) % 2 == 0:
                            nc.scalar.copy(dst, wide3(pt))
                        else:
                            nc.vector.tensor_copy(dst, wide3(pt))

            def emit_ffn_half(h2):
                """FFN on tokens [h2*256, h2*256+256) of this quarter."""
                c0 = h2 * 256
                for n in range(KC):
                    ps = ps512.tile([P, 256], f32, name=f"ps1_{q}_{h2}_{n}",
                                    tag="tr")
                    for k in range(KC):
                        nc.tensor.matmul(ps, w1_sb[k][:, n * P:(n + 1) * P],
                                         lnT[:, k * QW + c0:k * QW + c0 + 256],
                                         start=(k == 0), stop=(k == KC - 1))
                    nc.scalar.activation(r1T[:, n * QW + c0:n * QW + c0 + 256],
                                         ps, Act.Relu, bias=b1_sb[:, n:n + 1])
                def emit_fnat_tr(dch):
                    # ffn back to natural layout, regrouped per token tile.
                    # Deferred one dch so the transpose's LDWEIGHTS never
                    # catches up to the DVE stt producing ffnT.
                    pt = ps512.tile([P, 256], f16, name=f"ptf{q}_{h2}_{dch}",
                                    tag="tr")
                    for tt in range(2):
                        ti = 2 * h2 + tt
                        nc.tensor.transpose(
                            pt[:, tt * P:(tt + 1) * P],
                            ffnT[:, dch * QW + ti * P:dch * QW + (ti + 1) * P],
                            ident16)
                    nc.scalar.copy(fv[:, 2 * h2:2 * h2 + 2,
                                      dch * P:(dch + 1) * P], wide3(pt))

                for dch in range(KC):
                    ps = ps512.tile([P, 256], f32, name=f"ps2_{q}_{h2}_{dch}",
                                    tag="tr")
                    for k in range(KC):
                        nc.tensor.matmul(ps, w2_sb[k][:, dch * P:(dch + 1) * P],
                                         r1T[:, k * QW + c0:k * QW + c0 + 256],
                                         start=(k == 0), stop=(k == KC - 1))
                    if dch > 0:
                        emit_fnat_tr(dch - 1)
                    # ffnT = (y2T + b2) + avgT  (f16 out)
                    sl = slice(dch * QW + c0, dch * QW + c0 + 256)
                    nc.vector.scalar_tensor_tensor(
                        ffnT[:, sl], ps, b2_sb[:, dch:dch + 1],
                        avgT[:, sl], op0=Alu.add, op1=Alu.add)
                    # fp8 shadow for the gating matmul (chunks 8..15); DVE --
                    # this feeds phase C's matmuls, and GPSIMD is ~3x slower
                    nc.vector.tensor_copy(catfv[:, dch, c0:c0 + 256],
                                          ffnT[:, sl])
                emit_fnat_tr(KC - 1)
                for tt in range(2):
                    ti = 2 * h2 + tt
                    i = q * QT + ti
                    nc.sync.dma_start(out=ffn_d[i * P:(i + 1) * P, :],
                                      in_=fnat[:, ti * D:(ti + 1) * D])

            def emit_gate_tile(ti):
                """Phase C for one token tile: gating matmuls + epilogue."""
                i = q * QT + ti
                last = (i == NT - 1)
                prods = []
                for h in range(2):  # 0: input gate (j 0..1023), 1: forget gate
                    gps = gate_p.tile([P, D], f32, name=f"gps_{i}_{h}", tag="g")
                    for cp in range(NCP):
                        if cp < 4:
                            lhsT = catxv[:, 2 * cp:2 * cp + 2,
                                         ti * P:(ti + 1) * P]
                        elif cp < 8:
                            lhsT = catfv[:, 2 * (cp - 4):2 * (cp - 4) + 2,
                                         ti * P:(ti + 1) * P]
                        else:
                            lhsT = o2v
                        for jb in range(4):
                            j0 = h * D + jb * 256
                            # start only on the first write into each 2KB PSUM
                            # bank (start marks the whole bank pending-zero)
                            nc.tensor.matmul(gps[:, jb * 256:(jb + 1) * 256],
                                             lhsT,
                                             gwv[:, 2 * cp:2 * cp + 2, j0:j0 + 256],
                                             start=(cp == 0 and jb % 2 == 0),
                                             stop=(cp == NCP - 1),
                                             perf_mode=DR, skip_group_check=True)
                    sig = sigp.tile([P, D], f16, name=f"sig_{i}_{h}", tag="sig")
                    nc.scalar.activation(sig, gps, Act.Sigmoid)
                    # sig_ig*x on GPSIMD (idle engine), sig_fg*ffn on DVE;
                    # all-DVE on the last tile so the kernel tail is short
                    src = xi_tiles[ti] if h == 0 else fnat[:, ti * D:(ti + 1) * D]
                    prod = tmpp.tile([P, D], f16, name=f"prod_{i}_{h}",
                                     tag=f"prod{h}")
                    eng = nc.gpsimd if (h == 0 and not last) else nc.vector
                    eng.tensor_tensor(prod, sig, src, op=Alu.mult)
                    prods.append(prod)
                o = tmpp.tile([P, D], f16, name=f"o_{i}", tag="o")
                nc.vector.tensor_tensor(o, prods[0], prods[1], op=Alu.add)
                nc.sync.dma_start(out=out_d[i * P:(i + 1) * P, :], in_=o)

            # ---- phases A+B+C interleaved; stril(t2)/stril(t3) are deferred
            # ---- past B0 so B0's matmuls cover the psA readouts; C(t0,t1)
            # ---- run between the FFN halves so their epilogues overlap B1
            a0 = emit_prefix(0)
            emit_arest(0, a0)
            a1 = emit_prefix(1)
            emit_arest(1, a1)
            if q == 0:
                emit_w2_dmas()
            a2 = emit_prefix(2)
            emit_lntr_pair(0, 1)
            flush_avgtr()        # avgT(1), needed by ffn_half(0)'s stt
            emit_ffn_half(0)
            emit_arest(2, a2)
            a3 = emit_prefix(3)
            emit_arest(3, a3)
            if q == 0:
                emit_gw8_dmas()
                emit_catx_dmas(0, catx_cur)
            # prefetch next quarter's x tiles on sync ahead of the out stores
            if q + 1 < NQ:
                xq_next = []
                for ti in range(QT):
                    i2 = (q + 1) * QT + ti
                    xi = xload.tile([P, D], f16, name=f"xi_{i2}", tag="xi")
                    nc.sync.dma_start(out=xi, in_=x_d[i2 * P:(i2 + 1) * P, :])
                    xq_next.append(xi)
                xq_cur = xq_next
                catx_next = quartA.tile([P, KC * QW], f8,
                                        name=f"catx_{q + 1}", tag="catx")
                emit_catx_dmas(q + 1, catx_next)
                catx_cur = catx_next
            emit_lntr_pair(2, 3)
            flush_avgtr()        # avgT(3), needed by ffn_half(1)'s stt
            if q == 0:
                # gw8 (4MB) cannot land before ~30us no matter the queue
                # order; defer quarter 0's gating past B1 so the PE never
                # stalls on it
                emit_ffn_half(1)
                for ti in range(QT):
                    emit_gate_tile(ti)
            else:
                emit_gate_tile(0)
                emit_gate_tile(1)
                emit_ffn_half(1)
                emit_gate_tile(2)
                emit_gate_tile(3)

    nc.compile()
    _CACHE[key] = nc
    return nc


def _prep_maps(inputs, ln_g, ln_b, w1, b1, w2, b2, gw, gb):
    import ml_dtypes

    inputs = np.asarray(inputs, dtype=np.float32)
    ln_g = np.asarray(ln_g, dtype=np.float32)
    ln_b = np.asarray(ln_b, dtype=np.float32)
    w1 = np.asarray(w1, dtype=np.float32)
    b1 = np.asarray(b1, dtype=np.float32)
    w2 = np.asarray(w2, dtype=np.float32)
    b2 = np.asarray(b2, dtype=np.float32)
    gw = np.asarray(gw, dtype=np.float32)
    gb = np.asarray(gb, dtype=np.float32)

    has_bias = bool(np.any(gb != 0.0))
    GCH = 18 if has_bias else 16

    w1f = (ln_g[:, None] * w1).astype(np.float32)
    b1f = (ln_b @ w1 + b1).astype(np.float32)

    # gating weights: rows 0..2047 = gw (+ row 2048 = gb when nonzero); fp8.
    # layout [k within chunk, chunk c, j] so gwv[:, c, j] = gw_ext[c*128+k, j]
    gw_ext = np.zeros((GCH * P, 2 * D), np.float32)
    gw_ext[:2 * D] = gw
    if has_bias:
        gw_ext[2 * D] = gb
    gw8 = np.ascontiguousarray(
        gw_ext.reshape(GCH, P, 2 * D).transpose(1, 0, 2)
    ).astype(ml_dtypes.float8_e4m3).reshape(P, GCH * 2 * D)

    base = {
        "w1": np.ascontiguousarray(w1f.reshape(KC, P, D)).astype(np.float16),
        "b1": np.ascontiguousarray(b1f.reshape(KC, P).T),
        "w2": np.ascontiguousarray(w2.reshape(KC, P, D)).astype(np.float16),
        "b2": np.ascontiguousarray(b2.reshape(KC, P).T),
        "gw8": gw8,
        "invsteps": np.ascontiguousarray(
            (1.0 / np.arange(1, L + 1, dtype=np.float32)).reshape(NT, P).T),
        "triu": np.triu(np.ones((P, P), np.float16)),
        "stril": np.tril(np.ones((P, P), np.float16), -1),
        "ident": np.eye(P, dtype=np.float32),
        "ident16": np.eye(P, dtype=np.float16),
    }
    if has_bias:
        ones2 = np.zeros((P, 2 * P), np.float32)
        ones2[0, 0:P] = 1.0
        base["ones2"] = ones2.astype(ml_dtypes.float8_e4m3)
    maps = []
    for b in range(B):
        x16 = np.ascontiguousarray(inputs[b]).astype(np.float16)
        # host-transposed fp8 x for the gating matmul's stationary operand;
        # same f16 -> fp8e4 rounding the device cast used
        xt8 = np.ascontiguousarray(
            x16.astype(ml_dtypes.float8_e4m3)
               .reshape(NQ, QW, KC, P).transpose(0, 2, 3, 1))
        maps.append(dict(base, x=x16, xt8=xt8))
    return maps, has_bias


def _run(in_maps, has_bias, trace=False):
    from concourse.bass_utils import run_bass_kernel_spmd
    nc = _build(has_bias)
    return run_bass_kernel_spmd(nc, in_maps, list(range(B)), trace=trace)


def kernel(inputs, ln_g, ln_b, w1, b1, w2, b2, gw, gb):
    in_maps, has_bias = _prep_maps(inputs, ln_g, ln_b, w1, b1, w2, b2, gw, gb)
    res = _run(in_maps, has_bias).results
    out = np.stack([res[b]["out"].astype(np.float32) for b in range(B)])
    ffn = np.stack([res[b]["ffn"].astype(np.float32) for b in range(B)])
    return out, ffn


def kernel_traced(inputs, ln_g, ln_b, w1, b1, w2, b2, gw, gb):
    """Like kernel(), but also returns the BassKernelResults (with exec_time_ns)."""
    in_maps, has_bias = _prep_maps(inputs, ln_g, ln_b, w1, b1, w2, b2, gw, gb)
    bkr = _run(in_maps, has_bias, trace=True)
    res = bkr.results
    out = np.stack([res[b]["out"].astype(np.float32) for b in range(B)])
    ffn = np.stack([res[b]["ffn"].astype(np.float32) for b in range(B)])
    return (out, ffn), bkr


# revision 34
# speedup vs baseline: 1.0082x; 1.0082x over previous
"""Bass/Tile TRN2 kernel for nn_AverageAttention (cumavg -> LN -> FFN -> sigmoid gating).

Sharding: data-parallel over batch, one batch element per NeuronCore (B=8, 8 cores).

Per-core pipeline (L=2048 tokens in 4 quarters of 512 = 4 tiles of 128):
  phase A (per 128-token tile, natural [t, d] layout):
     cumavg via triu-matmul + running-prefix (strict-lower-tril matmul) in a
     persistent PSUM region; avg scale split ACT/DVE; LayerNorm stats via
     bn_stats/bn_aggr with rstd computed on DVE via fast-inverse-sqrt
     (bit hack + 1 Newton step, batched per tile pair) so the ACT engine
     never loads the sqrt table -- the only table function left is Sigmoid
     (one ACT_TABLE_LOAD for the whole kernel instead of ~21).
     PE-transposes produce avgT (f16) / x-chunks of catq8 (fp8) in [d, t]
     layout; lnT transposes are batched per pair after the normalize.
  phase B (per quarter): y1T = w1'@lnT (relu+b1 on ACT, fp16), y2T = w2@r1T,
     ffnT = y2T + b2 + avgT (f16); ffnT cast into catq8 (fp8) on GPSIMD;
     ffnT transposed back to natural layout (fnat, f16) and DMA'd out per
     128-token row block (scalar queue).
  phase C (per 128-token tile, natural output layout): gating computed as
     gate[t, j] = sum_c catT8[c-pair]^T @ gw8[c-pair, j] with fp8 DoubleRow
     matmuls. Stationary = catq8 chunk-pair x t-tile, moving = gw8
     (SBUF-resident). sigmoid -> f16; the two sig*x / sig*ffn products run
     on the otherwise-idle GPSIMD engine (DVE for the last tile to shorten
     the tail), final add on DVE, out stored f16 on the vector queue.
     C tiles 0-1 of each quarter are emitted between the two FFN halves so
     their epilogues overlap B1's matmuls.

Weights stream on three DMA queues (scalar/vector/gpsimd) with w1 first so
phase B(0) is not gated on a single queue; consts+x ride sync. Outputs are
f16 (upcast to f32 on the host). ln_g/ln_b folded into w1/b1 on host.
"""

import numpy as np

B, L, D = 8, 2048, 1024
P = 128
NT = L // P          # 16 token tiles
KC = D // P          # 8 d-chunks
QT = 4               # tiles per quarter
NQ = NT // QT        # 4 quarters
QW = QT * P          # 512 tokens per quarter
EPS = 1e-6

_CACHE = {}


def _build(has_bias):
    key = ("nc", has_bias)
    if key in _CACHE:
        return _CACHE[key]

    import concourse.bacc as bacc
    import concourse.mybir as mybir
    import concourse.tile as tile
    from contextlib import ExitStack

    f32 = mybir.dt.float32
    f32r = mybir.dt.float32r
    f16 = mybir.dt.float16
    f8 = mybir.dt.float8e4
    i32 = mybir.dt.int32
    Alu = mybir.AluOpType
    Act = mybir.ActivationFunctionType
    DR = mybir.MatmulPerfMode.DoubleRow

    GCH = 18 if has_bias else 16   # gating contraction chunks
    NCP = GCH // 2                 # chunk pairs per gate psum fill

    nc = bacc.Bacc("TRN2", debug=False, target_bir_lowering=False, num_devices=B)

    x_d = nc.dram_tensor("x", [L, D], f16, kind="ExternalInput").ap()
    xt8_d = nc.dram_tensor("xt8", [NQ, KC, P, QW], f8, kind="ExternalInput").ap()
    w1_d = nc.dram_tensor("w1", [KC, P, D], f16, kind="ExternalInput").ap()
    b1_d = nc.dram_tensor("b1", [P, KC], f32, kind="ExternalInput").ap()
    w2_d = nc.dram_tensor("w2", [KC, P, D], f16, kind="ExternalInput").ap()
    b2_d = nc.dram_tensor("b2", [P, KC], f32, kind="ExternalInput").ap()
    gw8_d = nc.dram_tensor("gw8", [P, GCH * 2 * D], f8, kind="ExternalInput").ap()
    if has_bias:
        ones2_d = nc.dram_tensor("ones2", [P, 2 * P], f8, kind="ExternalInput").ap()
    inv_d = nc.dram_tensor("invsteps", [P, NT], f32, kind="ExternalInput").ap()
    triu_d = nc.dram_tensor("triu", [P, P], f16, kind="ExternalInput").ap()
    stril_d = nc.dram_tensor("stril", [P, P], f16, kind="ExternalInput").ap()
    ident_d = nc.dram_tensor("ident", [P, P], f32r, kind="ExternalInput").ap()
    ident16_d = nc.dram_tensor("ident16", [P, P], f16, kind="ExternalInput").ap()
    out_d = nc.dram_tensor("out", [L, D], f16, kind="ExternalOutput").ap()
    ffn_d = nc.dram_tensor("ffn", [L, D], f16, kind="ExternalOutput").ap()

    def r(ap):
        return ap.bitcast(f32r)

    def v(ap):
        return ap.bitcast(f32)

    def wide3(ap, inner=P):
        return ap.rearrange("p (b t) -> p b t", t=inner)

    with tile.TileContext(nc) as tc, ExitStack() as ctx:
        consts = ctx.enter_context(tc.tile_pool(name="consts", bufs=1))
        wts = ctx.enter_context(tc.tile_pool(name="wts", bufs=1))
        quartA = ctx.enter_context(tc.tile_pool(name="quartA", bufs=2))
        quartB = ctx.enter_context(tc.tile_pool(name="quartB", bufs=1))
        xload = ctx.enter_context(tc.tile_pool(name="xload", bufs=8))
        avgp = ctx.enter_context(tc.tile_pool(name="avgp", bufs=2))
        statp = ctx.enter_context(tc.tile_pool(name="statp", bufs=2))
        sigp = ctx.enter_context(tc.tile_pool(name="sigp", bufs=3))
        tmpp = ctx.enter_context(tc.tile_pool(name="tmpp", bufs=2))
        psA_p = ctx.enter_context(tc.tile_pool(name="psA", bufs=1, space="PSUM"))
        ps512 = ctx.enter_context(tc.tile_pool(name="ps512", bufs=2, space="PSUM"))
        gate_p = ctx.enter_context(tc.tile_pool(name="gate", bufs=2, space="PSUM"))

        # startup DMA layout. Only sync (SP) and scalar (ACT) rings have
        # hardware DGE -- gpsimd DMAs go through a slow software path, so the
        # gpsimd ring carries ONLY x0 + small consts (needed in the first
        # ~10us, tiny). All weights ride the two HW rings: w1 split across
        # both (phase B(0) needs it ~15us in), then w2/gw8. Small consts
        # never go ahead of x on a HW ring -- their 128-row tiny descriptors
        # starve the x loads for ~25us.
        xi_pre = [xload.tile([P, D], f16, name=f"xi_{i}", tag="xi")
                  for i in range(QT)]
        nc.gpsimd.dma_start(out=xi_pre[0], in_=x_d[0:P, :])
        triu = consts.tile([P, P], f16, name="triu_sb")
        nc.scalar.dma_start(out=triu, in_=triu_d)
        stril = consts.tile([P, P], f16, name="stril_sb")
        nc.scalar.dma_start(out=stril, in_=stril_d)
        ident16 = consts.tile([P, P], f16, name="ident16_sb")
        nc.scalar.dma_start(out=ident16, in_=ident16_d)
        for i in range(1, QT):
            nc.sync.dma_start(out=xi_pre[i], in_=x_d[i * P:(i + 1) * P, :])
        ident = consts.tile([P, P], f32r, name="ident_sb")
        nc.gpsimd.dma_start(out=ident, in_=ident_d)
        inv_sb = consts.tile([P, NT], f32, name="inv_sb")
        nc.gpsimd.dma_start(out=inv_sb, in_=inv_d)
        b1_sb = consts.tile([P, KC], f32, name="b1_sb")
        nc.gpsimd.dma_start(out=b1_sb, in_=b1_d)
        b2_sb = consts.tile([P, KC], f32, name="b2_sb")
        nc.gpsimd.dma_start(out=b2_sb, in_=b2_d)
        if has_bias:
            ones2 = consts.tile([P, 2 * P], f8, name="ones2_sb")
            nc.gpsimd.dma_start(out=ones2, in_=ones2_d)
            o2v = ones2.rearrange("p (s t) -> p s t", s=2)

        # DMA progress is shared across everything queued (descriptor-level
        # fair share), so w2/gw8 dma_starts are DEFERRED into the quarter-0
        # instruction stream: w1 gets the full pipe first and phase B(0)
        # starts ~15us earlier. Tiles are allocated here; dma_starts later.
        w1_sb = [None] * KC
        w2_sb = [None] * KC
        for k in range(KC):
            t1 = wts.tile([P, D], f16, name=f"w1sb{k}", tag=f"w1_{k}")
            (nc.scalar if k % 2 == 0 else nc.sync).dma_start(out=t1, in_=w1_d[k])
            w1_sb[k] = t1
        for k in range(KC):
            w2_sb[k] = wts.tile([P, D], f16, name=f"w2sb{k}", tag=f"w2_{k}")
        gw8 = wts.tile([P, GCH * 2 * D], f8, name="gw8_sb")
        gw_half = (GCH // 2) * 2 * D
        gwv = gw8.rearrange("p (c j) -> p c j", c=GCH)       # [P, GCH, 2048]

        def emit_w2_dmas():
            for k in range(KC):
                nc.scalar.dma_start(out=w2_sb[k], in_=w2_d[k])

        def emit_gw8_dmas():
            nc.sync.dma_start(out=gw8[:, 0:gw_half], in_=gw8_d[:, 0:gw_half])
            nc.scalar.dma_start(out=gw8[:, gw_half:], in_=gw8_d[:, gw_half:])

        # PE warmup: the HAM clock gate keeps the PE at 1.2 GHz until it sees
        # ~3.4us of sustained matmul activity, and transpose-mode work does
        # not count. Burn dummy matmuls on triu (first tensor to arrive)
        # while waiting for x0 so phase A runs at 2.4 GHz.
        for wu in range(3):
            scr = ps512.tile([P, 512], f32, name=f"warm_{wu}", tag="tr")
            for cc in range(16):
                nc.tensor.matmul(scr[:, (cc % 4) * P:(cc % 4 + 1) * P], triu,
                                 triu, start=(cc < 4), stop=(cc >= 12))

        # persistent PSUM region carrying the running column-sum prefix R
        psA = psA_p.tile([P, D], f32, name="psA_t")

        # x tiles and catx (host-transposed fp8 x) for quarter q are
        # prefetched during quarter q-1 (before its out stores hit the sync
        # queue, avoiding head-of-line blocking)
        xq_cur = xi_pre
        catx_cur = quartA.tile([P, KC * QW], f8, name="catx_0", tag="catx")

        def emit_catx_dmas(qq, tile_):
            cv = tile_.rearrange("p (c t) -> p c t", c=KC)
            for c in range(KC):
                eng = nc.sync if c % 2 == 0 else nc.scalar
                eng.dma_start(out=cv[:, c, :], in_=xt8_d[qq, c])

        for q in range(NQ):
            lnT = quartA.tile([P, KC * QW], f16, name=f"lnT_{q}", tag="lnT")
            avgT = quartA.tile([P, KC * QW], f16, name=f"avgT_{q}", tag="avgT")
            catx8 = catx_cur
            catxv = catx8.rearrange("p (c t) -> p c t", c=KC)
            catf8 = quartA.tile([P, KC * QW], f8, name=f"catf_{q}", tag="catf")
            catfv = catf8.rearrange("p (c t) -> p c t", c=KC)
            r1T = quartB.tile([P, KC * QW], f16, name=f"r1T_{q}", tag="r1T")
            ffnT = quartB.tile([P, KC * QW], f16, name=f"ffnT_{q}", tag="ffnT")
            fnat = quartB.tile([P, QT * D], f16, name=f"fnat_{q}", tag="fnat")
            fv = fnat.rearrange("p (t d) -> p t d", t=QT)
            xi_tiles = []
            avgs = {}
            lns = {}
            avg_pend = [None]

            def emit_prefix(ti):
                """x load + triu cumsum + psA readout (avg scale) for tile ti.

                Emitted as early as possible so the ACT/DVE psA reads overlap
                whatever PE work precedes the strict-tril update."""
                i = q * QT + ti
                xi = xq_cur[ti]
                xi_tiles.append(xi)
                for s in range(2):
                    nc.tensor.matmul(psA[:, s * 512:(s + 1) * 512], triu,
                                     xi[:, s * 512:(s + 1) * 512],
                                     start=(i == 0), stop=False)
                avg_i = avgp.tile([P, D], f32r, name=f"avg_{i}", tag="avg")
                nc.scalar.mul(avg_i[:, 0:512], psA[:, 0:512], inv_sb[:, i:i + 1])
                nc.vector.tensor_scalar_mul(avg_i[:, 512:1024], psA[:, 512:1024],
                                            inv_sb[:, i:i + 1])
                return avg_i

            def flush_avgtr():
                """Deferred avg->avgT transposes of the previous tile; called
                at the top of arest so they fill the PE bubble while ACT/DVE
                read psA out (the role x-transposes played before catx moved
                to a host-side DMA)."""
                if avg_pend[0] is None:
                    return
                ti_, avg_, avgT_ = avg_pend[0]
                avg_pend[0] = None
                for g in range(2):
                    pt = ps512.tile([P, 512], f32, name=f"pta{q}_{ti_}_{g}",
                                    tag="tr")
                    for cc in range(4):
                        c = g * 4 + cc
                        nc.tensor.transpose(r(pt[:, cc * P:(cc + 1) * P]),
                                            avg_[:, c * P:(c + 1) * P], ident)
                    dst = wide3(avgT_, QW)[:, g * 4:(g + 1) * 4,
                                           ti_ * P:(ti_ + 1) * P]
                    nc.scalar.copy(dst, wide3(pt))

            def emit_arest(ti, avg_i):
                i = q * QT + ti
                xi = xi_tiles[ti]
                avgs[ti] = avg_i
                # PE bubble filler while ACT/DVE read psA out
                flush_avgtr()
                # psA += strict-lower-tril(x_i)  (now holds R_{i+1})
                for s in range(2):
                    nc.tensor.matmul(psA[:, s * 512:(s + 1) * 512], stril,
                                     xi[:, s * 512:(s + 1) * 512],
                                     start=False, stop=(i == NT - 1))

                # LN stats on avg_i, then per-tile rstd (fast-inverse-sqrt
                # seed + one Newton step on DVE -- no ACT sqrt table) and the
                # normalize, so each tile's chain hides in its own slack
                st6 = statp.tile([P, 12], f32, name=f"st6_{i}", tag="st6")
                nc.vector.bn_stats(st6[:, 0:6], v(avg_i[:, 0:512]))
                nc.vector.bn_stats(st6[:, 6:12], v(avg_i[:, 512:1024]))
                mv = statp.tile([P, 2], f32, name=f"mv_{i}", tag="mv")
                nc.vector.bn_aggr(mv, st6.rearrange("p (g s) -> p g s", g=2))
                avg_pend[0] = (ti, avg_i, avgT)

                ve = statp.tile([P, 1], f32, name=f"ve_{i}", tag="ve")
                nc.vector.tensor_scalar(ve, mv[:, 1:2], EPS, None, op0=Alu.add)
                sd = statp.tile([P, 1], f32, name=f"sd_{i}", tag="sd")
                nc.vector.tensor_scalar(sd.bitcast(i32), ve.bitcast(i32),
                                        1, 0xFFFFFFFF,
                                        op0=Alu.logical_shift_right,
                                        op1=Alu.bitwise_xor)
                nc.vector.tensor_scalar(sd.bitcast(i32), sd.bitcast(i32),
                                        0x5F3759E0, None, op0=Alu.add)
                hh = statp.tile([P, 1], f32, name=f"hh_{i}", tag="hh")
                nc.vector.tensor_tensor(hh, sd, sd, op=Alu.mult)
                nc.vector.tensor_tensor(hh, hh, ve, op=Alu.mult)
                nc.vector.tensor_scalar(hh, hh, -0.5, 1.5,
                                        op0=Alu.mult, op1=Alu.add)
                nc.vector.tensor_tensor(sd, sd, hh, op=Alu.mult)
                # ln = (avg - mean) * rstd into a separate buffer (avg_i
                # stays raw for the deferred avgT transposes)
                ln_i = avgp.tile([P, D], f32r, name=f"ln_{i}", tag="ln")
                nc.vector.tensor_scalar(ln_i, v(avg_i), mv[:, 0:1], sd,
                                        op0=Alu.subtract, op1=Alu.mult)
                lns[ti] = ln_i

            def emit_lntr_pair(t0, t1):
                """lnT transposes for a normalized tile pair (PE + ACT/DVE)."""
                for idx, ti_ in enumerate((t0, t1)):
                    avg_ = lns[ti_]
                    for g in range(2):
                        pt = ps512.tile([P, 512], f32, name=f"ptl{q}_{ti_}_{g}",
                                        tag="tr")
                        for cc in range(4):
                            c = g * 4 + cc
                            nc.tensor.transpose(r(pt[:, cc * P:(cc + 1) * P]),
                                                avg_[:, c * P:(c + 1) * P], ident)
                        dst = wide3(lnT, QW)[:, g * 4:(g + 1) * 4,
                                             ti_ * P:(ti_ + 1) * P]
                        if (idx + g) % 2 == 0:
                            nc.scalar.copy(dst, wide3(pt))
                        else:
                            nc.vector.tensor_copy(dst, wide3(pt))

            def emit_ffn_half(h2):
                """FFN on tokens [h2*256, h2*256+256) of this quarter."""
                c0 = h2 * 256
                for n in range(KC):
                    ps = ps512.tile([P, 256], f32, name=f"ps1_{q}_{h2}_{n}",
                                    tag="tr")
                    for k in range(KC):
                        nc.tensor.matmul(ps, w1_sb[k][:, n * P:(n + 1) * P],
                                         lnT[:, k * QW + c0:k * QW + c0 + 256],
                                         start=(k == 0), stop=(k == KC - 1))
                    nc.scalar.activation(r1T[:, n * QW + c0:n * QW + c0 + 256],
                                         ps, Act.Relu, bias=b1_sb[:, n:n + 1])
                def emit_fnat_tr(dch):
                    # ffn back to natural layout, regrouped per token tile.
                    # Deferred one dch so the transpose's LDWEIGHTS never
                    # catches up to the DVE stt producing ffnT.
                    pt = ps512.tile([P, 256], f16, name=f"ptf{q}_{h2}_{dch}",
                                    tag="tr")
                    for tt in range(2):
                        ti = 2 * h2 + tt
                        nc.tensor.transpose(
                            pt[:, tt * P:(tt + 1) * P],
                            ffnT[:, dch * QW + ti * P:dch * QW + (ti + 1) * P],
                            ident16)
                    dst = fv[:, 2 * h2:2 * h2 + 2, dch * P:(dch + 1) * P]
                    if dch % 2 == 0:
                        nc.scalar.copy(dst, wide3(pt))
                    else:
                        nc.vector.tensor_copy(dst, wide3(pt))

                for dch in range(KC):
                    ps = ps512.tile([P, 256], f32, name=f"ps2_{q}_{h2}_{dch}",
                                    tag="tr")
                    for k in range(KC):
                        nc.tensor.matmul(ps, w2_sb[k][:, dch * P:(dch + 1) * P],
                                         r1T[:, k * QW + c0:k * QW + c0 + 256],
                                         start=(k == 0), stop=(k == KC - 1))
                    if dch > 0:
                        emit_fnat_tr(dch - 1)
                    # ffnT = (y2T + b2) + avgT  (f16 out)
                    sl = slice(dch * QW + c0, dch * QW + c0 + 256)
                    nc.vector.scalar_tensor_tensor(
                        ffnT[:, sl], ps, b2_sb[:, dch:dch + 1],
                        avgT[:, sl], op0=Alu.add, op1=Alu.add)
                    # fp8 shadow for the gating matmul (chunks 8..15); DVE --
                    # this feeds phase C's matmuls, and GPSIMD is ~3x slower
                    nc.vector.tensor_copy(catfv[:, dch, c0:c0 + 256],
                                          ffnT[:, sl])
                emit_fnat_tr(KC - 1)
                for tt in range(2):
                    ti = 2 * h2 + tt
                    i = q * QT + ti
                    nc.sync.dma_start(out=ffn_d[i * P:(i + 1) * P, :],
                                      in_=fnat[:, ti * D:(ti + 1) * D])

            def emit_gate_tile(ti):
                """Phase C for one token tile: gating matmuls + epilogue."""
                i = q * QT + ti
                last = (i == NT - 1)
                prods = []
                for h in range(2):  # 0: input gate (j 0..1023), 1: forget gate
                    gps = gate_p.tile([P, D], f32, name=f"gps_{i}_{h}", tag="g")
                    for cp in range(NCP):
                        if cp < 4:
                            lhsT = catxv[:, 2 * cp:2 * cp + 2,
                                         ti * P:(ti + 1) * P]
                        elif cp < 8:
                            lhsT = catfv[:, 2 * (cp - 4):2 * (cp - 4) + 2,
                                         ti * P:(ti + 1) * P]
                        else:
                            lhsT = o2v
                        for jb in range(4):
                            j0 = h * D + jb * 256
                            # start only on the first write into each 2KB PSUM
                            # bank (start marks the whole bank pending-zero)
                            nc.tensor.matmul(gps[:, jb * 256:(jb + 1) * 256],
                                             lhsT,
                                             gwv[:, 2 * cp:2 * cp + 2, j0:j0 + 256],
                                             start=(cp == 0 and jb % 2 == 0),
                                             stop=(cp == NCP - 1),
                                             perf_mode=DR, skip_group_check=True)
                    sig = sigp.tile([P, D], f16, name=f"sig_{i}_{h}", tag="sig")
                    nc.scalar.activation(sig, gps, Act.Sigmoid)
                    # sig_ig*x on GPSIMD (idle engine), sig_fg*ffn on DVE;
                    # all-DVE on the last tile so the kernel tail is short
                    src = xi_tiles[ti] if h == 0 else fnat[:, ti * D:(ti + 1) * D]
                    prod = tmpp.tile([P, D], f16, name=f"prod_{i}_{h}",
                                     tag=f"prod{h}")
                    eng = nc.gpsimd if (h == 0 and not last) else nc.vector
                    eng.tensor_tensor(prod, sig, src, op=Alu.mult)
                    prods.append(prod)
                o = tmpp.tile([P, D], f16, name=f"o_{i}", tag="o")
                nc.vector.tensor_tensor(o, prods[0], prods[1], op=Alu.add)
                nc.sync.dma_start(out=out_d[i * P:(i + 1) * P, :], in_=o)

            # ---- phases A+B+C interleaved; stril(t2)/stril(t3) are deferred
            # ---- past B0 so B0's matmuls cover the psA readouts; C(t0,t1)
            # ---- run between the FFN halves so their epilogues overlap B1
            a0 = emit_prefix(0)
            emit_arest(0, a0)
            a1 = emit_prefix(1)
            emit_arest(1, a1)
            if q == 0:
                emit_w2_dmas()
            a2 = emit_prefix(2)
            emit_lntr_pair(0, 1)
            flush_avgtr()        # avgT(1), needed by ffn_half(0)'s stt
            emit_ffn_half(0)
            emit_arest(2, a2)
            a3 = emit_prefix(3)
            emit_arest(3, a3)
            if q == 0:
                emit_gw8_dmas()
                emit_catx_dmas(0, catx_cur)
            # prefetch next quarter's x tiles on sync ahead of the out stores
            if q + 1 < NQ:
                xq_next = []
                for ti in range(QT):
                    i2 = (q + 1) * QT + ti
                    xi = xload.tile([P, D], f16, name=f"xi_{i2}", tag="xi")
                    nc.sync.dma_start(out=xi, in_=x_d[i2 * P:(i2 + 1) * P, :])
                    xq_next.append(xi)
                xq_cur = xq_next
                catx_next = quartA.tile([P, KC * QW], f8,
                                        name=f"catx_{q + 1}", tag="catx")
                emit_catx_dmas(q + 1, catx_next)
                catx_cur = catx_next
            emit_lntr_pair(2, 3)
            flush_avgtr()        # avgT(3), needed by ffn_half(1)'s stt
            if q == 0:
                # gw8 (4MB) cannot land before ~30us no matter the queue
                # order; defer quarter 0's gating past B1 so the PE never
                # stalls on it
                emit_ffn_half(1)
                for ti in range(QT):
                    emit_gate_tile(ti)
            else:
                emit_gate_tile(0)
                emit_gate_tile(1)
                emit_ffn_half(1)
                emit_gate_tile(2)
                emit_gate_tile(3)

    nc.compile()
    _CACHE[key] = nc
    return nc


def _prep_maps(inputs, ln_g, ln_b, w1, b1, w2, b2, gw, gb):
    import ml_dtypes

    inputs = np.asarray(inputs, dtype=np.float32)
    ln_g = np.asarray(ln_g, dtype=np.float32)
    ln_b = np.asarray(ln_b, dtype=np.float32)
    w1 = np.asarray(w1, dtype=np.float32)
    b1 = np.asarray(b1, dtype=np.float32)
    w2 = np.asarray(w2, dtype=np.float32)
    b2 = np.asarray(b2, dtype=np.float32)
    gw = np.asarray(gw, dtype=np.float32)
    gb = np.asarray(gb, dtype=np.float32)

    has_bias = bool(np.any(gb != 0.0))
    GCH = 18 if has_bias else 16

    w1f = (ln_g[:, None] * w1).astype(np.float32)
    b1f = (ln_b @ w1 + b1).astype(np.float32)

    # gating weights: rows 0..2047 = gw (+ row 2048 = gb when nonzero); fp8.
    # layout [k within chunk, chunk c, j] so gwv[:, c, j] = gw_ext[c*128+k, j]
    gw_ext = np.zeros((GCH * P, 2 * D), np.float32)
    gw_ext[:2 * D] = gw
    if has_bias:
        gw_ext[2 * D] = gb
    gw8 = np.ascontiguousarray(
        gw_ext.reshape(GCH, P, 2 * D).transpose(1, 0, 2)
    ).astype(ml_dtypes.float8_e4m3).reshape(P, GCH * 2 * D)

    base = {
        "w1": np.ascontiguousarray(w1f.reshape(KC, P, D)).astype(np.float16),
        "b1": np.ascontiguousarray(b1f.reshape(KC, P).T),
        "w2": np.ascontiguousarray(w2.reshape(KC, P, D)).astype(np.float16),
        "b2": np.ascontiguousarray(b2.reshape(KC, P).T),
        "gw8": gw8,
        "invsteps": np.ascontiguousarray(
            (1.0 / np.arange(1, L + 1, dtype=np.float32)).reshape(NT, P).T),
        "triu": np.triu(np.ones((P, P), np.float16)),
        "stril": np.tril(np.ones((P, P), np.float16), -1),
        "ident": np.eye(P, dtype=np.float32),
        "ident16": np.eye(P, dtype=np.float16),
    }
    if has_bias:
        ones2 = np.zeros((P, 2 * P), np.float32)
        ones2[0, 0:P] = 1.0
        base["ones2"] = ones2.astype(ml_dtypes.float8_e4m3)
    maps = []
    for b in range(B):
        x16 = np.ascontiguousarray(inputs[b]).astype(np.float16)
        # host-transposed fp8 x for the gating matmul's stationary operand;
        # same f16 -> fp8e4 rounding the device cast used
        xt8 = np.ascontiguousarray(
            x16.astype(ml_dtypes.float8_e4m3)
               .reshape(NQ, QW, KC, P).transpose(0, 2, 3, 1))
        maps.append(dict(base, x=x16, xt8=xt8))
    return maps, has_bias


def _run(in_maps, has_bias, trace=False):
    from concourse.bass_utils import run_bass_kernel_spmd
    nc = _build(has_bias)
    return run_bass_kernel_spmd(nc, in_maps, list(range(B)), trace=trace)


def kernel(inputs, ln_g, ln_b, w1, b1, w2, b2, gw, gb):
    in_maps, has_bias = _prep_maps(inputs, ln_g, ln_b, w1, b1, w2, b2, gw, gb)
    res = _run(in_maps, has_bias).results
    out = np.stack([res[b]["out"].astype(np.float32) for b in range(B)])
    ffn = np.stack([res[b]["ffn"].astype(np.float32) for b in range(B)])
    return out, ffn


def kernel_traced(inputs, ln_g, ln_b, w1, b1, w2, b2, gw, gb):
    """Like kernel(), but also returns the BassKernelResults (with exec_time_ns)."""
    in_maps, has_bias = _prep_maps(inputs, ln_g, ln_b, w1, b1, w2, b2, gw, gb)
    bkr = _run(in_maps, has_bias, trace=True)
    res = bkr.results
    out = np.stack([res[b]["out"].astype(np.float32) for b in range(B)])
    ffn = np.stack([res[b]["ffn"].astype(np.float32) for b in range(B)])
    return (out, ffn), bkr


# revision 37
# speedup vs baseline: 1.0200x; 1.0118x over previous
"""Bass/Tile TRN2 kernel for nn_AverageAttention (cumavg -> LN -> FFN -> sigmoid gating).

Sharding: data-parallel over batch, one batch element per NeuronCore (B=8, 8 cores).

Per-core pipeline (L=2048 tokens in 4 quarters of 512 = 4 tiles of 128):
  phase A (per 128-token tile, natural [t, d] layout):
     cumavg via triu-matmul + running-prefix (strict-lower-tril matmul) in a
     persistent PSUM region; avg scale split ACT/DVE; LayerNorm stats via
     bn_stats/bn_aggr with rstd computed on DVE via fast-inverse-sqrt
     (bit hack + 1 Newton step, batched per tile pair) so the ACT engine
     never loads the sqrt table -- the only table function left is Sigmoid
     (one ACT_TABLE_LOAD for the whole kernel instead of ~21).
     PE-transposes produce avgT (f16) / x-chunks of catq8 (fp8) in [d, t]
     layout; lnT transposes are batched per pair after the normalize.
  phase B (per quarter): y1T = w1'@lnT (relu+b1 on ACT, fp16), y2T = w2@r1T,
     ffnT = y2T + b2 + avgT (f16); ffnT cast into catq8 (fp8) on GPSIMD;
     ffnT transposed back to natural layout (fnat, f16) and DMA'd out per
     128-token row block (scalar queue).
  phase C (per 128-token tile, natural output layout): gating computed as
     gate[t, j] = sum_c catT8[c-pair]^T @ gw8[c-pair, j] with fp8 DoubleRow
     matmuls. Stationary = catq8 chunk-pair x t-tile, moving = gw8
     (SBUF-resident). sigmoid -> f16; the two sig*x / sig*ffn products run
     on the otherwise-idle GPSIMD engine (DVE for the last tile to shorten
     the tail), final add on DVE, out stored f16 on the vector queue.
     C tiles 0-1 of each quarter are emitted between the two FFN halves so
     their epilogues overlap B1's matmuls.

Weights stream on three DMA queues (scalar/vector/gpsimd) with w1 first so
phase B(0) is not gated on a single queue; consts+x ride sync. Outputs are
f16 (upcast to f32 on the host). ln_g/ln_b folded into w1/b1 on host.
"""

import numpy as np

B, L, D = 8, 2048, 1024
P = 128
NT = L // P          # 16 token tiles
KC = D // P          # 8 d-chunks
QT = 4               # tiles per quarter
NQ = NT // QT        # 4 quarters
QW = QT * P          # 512 tokens per quarter
EPS = 1e-6

_CACHE = {}


def _build(has_bias):
    key = ("nc", has_bias)
    if key in _CACHE:
        return _CACHE[key]

    import concourse.bacc as bacc
    import concourse.mybir as mybir
    import concourse.tile as tile
    from contextlib import ExitStack

    f32 = mybir.dt.float32
    f32r = mybir.dt.float32r
    f16 = mybir.dt.float16
    f8 = mybir.dt.float8e4
    i32 = mybir.dt.int32
    Alu = mybir.AluOpType
    Act = mybir.ActivationFunctionType
    DR = mybir.MatmulPerfMode.DoubleRow

    GCH = 18 if has_bias else 16   # gating contraction chunks
    NCP = GCH // 2                 # chunk pairs per gate psum fill

    nc = bacc.Bacc("TRN2", debug=False, target_bir_lowering=False, num_devices=B)

    x_d = nc.dram_tensor("x", [L, D], f16, kind="ExternalInput").ap()
    xt8_d = nc.dram_tensor("xt8", [NQ, KC, P, QW], f8, kind="ExternalInput").ap()
    w1_d = nc.dram_tensor("w1", [KC, P, D], f16, kind="ExternalInput").ap()
    b1_d = nc.dram_tensor("b1", [P, KC], f32, kind="ExternalInput").ap()
    w2_d = nc.dram_tensor("w2", [KC, P, D], f16, kind="ExternalInput").ap()
    b2_d = nc.dram_tensor("b2", [P, KC], f32, kind="ExternalInput").ap()
    gw8_d = nc.dram_tensor("gw8", [P, GCH * 2 * D], f8, kind="ExternalInput").ap()
    if has_bias:
        ones2_d = nc.dram_tensor("ones2", [P, 2 * P], f8, kind="ExternalInput").ap()
    inv_d = nc.dram_tensor("invsteps", [P, NT], f32, kind="ExternalInput").ap()
    triu_d = nc.dram_tensor("triu", [P, P], f16, kind="ExternalInput").ap()
    stril_d = nc.dram_tensor("stril", [P, P], f16, kind="ExternalInput").ap()
    ident_d = nc.dram_tensor("ident", [P, P], f32r, kind="ExternalInput").ap()
    ident16_d = nc.dram_tensor("ident16", [P, P], f16, kind="ExternalInput").ap()
    out_d = nc.dram_tensor("out", [L, D], f16, kind="ExternalOutput").ap()
    ffn_d = nc.dram_tensor("ffn", [L, D], f16, kind="ExternalOutput").ap()

    def r(ap):
        return ap.bitcast(f32r)

    def v(ap):
        return ap.bitcast(f32)

    def wide3(ap, inner=P):
        return ap.rearrange("p (b t) -> p b t", t=inner)

    with tile.TileContext(nc) as tc, ExitStack() as ctx:
        consts = ctx.enter_context(tc.tile_pool(name="consts", bufs=1))
        wts = ctx.enter_context(tc.tile_pool(name="wts", bufs=1))
        quartA = ctx.enter_context(tc.tile_pool(name="quartA", bufs=2))
        quartB = ctx.enter_context(tc.tile_pool(name="quartB", bufs=1))
        xload = ctx.enter_context(tc.tile_pool(name="xload", bufs=8))
        avgp = ctx.enter_context(tc.tile_pool(name="avgp", bufs=2))
        statp = ctx.enter_context(tc.tile_pool(name="statp", bufs=2))
        sigp = ctx.enter_context(tc.tile_pool(name="sigp", bufs=3))
        tmpp = ctx.enter_context(tc.tile_pool(name="tmpp", bufs=2))
        psA_p = ctx.enter_context(tc.tile_pool(name="psA", bufs=1, space="PSUM"))
        ps512 = ctx.enter_context(tc.tile_pool(name="ps512", bufs=2, space="PSUM"))
        gate_p = ctx.enter_context(tc.tile_pool(name="gate", bufs=2, space="PSUM"))

        # startup DMA layout. Only sync (SP) and scalar (ACT) rings have
        # hardware DGE -- gpsimd DMAs go through a slow software path, so the
        # gpsimd ring carries ONLY x0 + small consts (needed in the first
        # ~10us, tiny). All weights ride the two HW rings: w1 split across
        # both (phase B(0) needs it ~15us in), then w2/gw8. Small consts
        # never go ahead of x on a HW ring -- their 128-row tiny descriptors
        # starve the x loads for ~25us.
        xi_pre = [xload.tile([P, D], f16, name=f"xi_{i}", tag="xi")
                  for i in range(QT)]
        nc.gpsimd.dma_start(out=xi_pre[0], in_=x_d[0:P, :])
        triu = consts.tile([P, P], f16, name="triu_sb")
        nc.scalar.dma_start(out=triu, in_=triu_d)
        stril = consts.tile([P, P], f16, name="stril_sb")
        nc.scalar.dma_start(out=stril, in_=stril_d)
        ident16 = consts.tile([P, P], f16, name="ident16_sb")
        nc.scalar.dma_start(out=ident16, in_=ident16_d)
        for i in range(1, QT):
            nc.sync.dma_start(out=xi_pre[i], in_=x_d[i * P:(i + 1) * P, :])
        ident = consts.tile([P, P], f32r, name="ident_sb")
        nc.gpsimd.dma_start(out=ident, in_=ident_d)
        inv_sb = consts.tile([P, NT], f32, name="inv_sb")
        nc.gpsimd.dma_start(out=inv_sb, in_=inv_d)
        b1_sb = consts.tile([P, KC], f32, name="b1_sb")
        nc.gpsimd.dma_start(out=b1_sb, in_=b1_d)
        b2_sb = consts.tile([P, KC], f32, name="b2_sb")
        nc.gpsimd.dma_start(out=b2_sb, in_=b2_d)
        if has_bias:
            ones2 = consts.tile([P, 2 * P], f8, name="ones2_sb")
            nc.gpsimd.dma_start(out=ones2, in_=ones2_d)
            o2v = ones2.rearrange("p (s t) -> p s t", s=2)

        # DMA progress is shared across everything queued (descriptor-level
        # fair share), so w2/gw8 dma_starts are DEFERRED into the quarter-0
        # instruction stream: w1 gets the full pipe first and phase B(0)
        # starts ~15us earlier. Tiles are allocated here; dma_starts later.
        w1_sb = [None] * KC
        w2_sb = [None] * KC
        for k in range(KC):
            t1 = wts.tile([P, D], f16, name=f"w1sb{k}", tag=f"w1_{k}")
            (nc.scalar if k % 2 == 0 else nc.sync).dma_start(out=t1, in_=w1_d[k])
            w1_sb[k] = t1
        for k in range(KC):
            w2_sb[k] = wts.tile([P, D], f16, name=f"w2sb{k}", tag=f"w2_{k}")
        gw8 = wts.tile([P, GCH * 2 * D], f8, name="gw8_sb")
        gw_half = (GCH // 2) * 2 * D
        gwv = gw8.rearrange("p (c j) -> p c j", c=GCH)       # [P, GCH, 2048]

        def emit_w2_dmas():
            for k in range(KC):
                nc.scalar.dma_start(out=w2_sb[k], in_=w2_d[k])

        def emit_gw8_dmas():
            nc.sync.dma_start(out=gw8[:, 0:gw_half], in_=gw8_d[:, 0:gw_half])
            nc.scalar.dma_start(out=gw8[:, gw_half:], in_=gw8_d[:, gw_half:])

        # PE warmup: the HAM clock gate keeps the PE at 1.2 GHz until it sees
        # ~3.4us of sustained matmul activity, and transpose-mode work does
        # not count. Burn dummy matmuls on triu (first tensor to arrive)
        # while waiting for x0 so phase A runs at 2.4 GHz.
        for wu in range(3):
            scr = ps512.tile([P, 512], f32, name=f"warm_{wu}", tag="tr")
            for cc in range(16):
                nc.tensor.matmul(scr[:, (cc % 4) * P:(cc % 4 + 1) * P], triu,
                                 triu, start=(cc < 4), stop=(cc >= 12))

        # persistent PSUM region carrying the running column-sum prefix R
        psA = psA_p.tile([P, D], f32, name="psA_t")

        # x tiles and catx (host-transposed fp8 x) for quarter q are
        # prefetched during quarter q-1 (before its out stores hit the sync
        # queue, avoiding head-of-line blocking)
        xq_cur = xi_pre
        catx_cur = quartA.tile([P, KC * QW], f8, name="catx_0", tag="catx")

        def emit_catx_dmas(qq, tile_):
            cv = tile_.rearrange("p (c t) -> p c t", c=KC)
            for c in range(KC):
                eng = nc.sync if c % 2 == 0 else nc.scalar
                eng.dma_start(out=cv[:, c, :], in_=xt8_d[qq, c])

        for q in range(NQ):
            lnT = quartA.tile([P, KC * QW], f16, name=f"lnT_{q}", tag="lnT")
            avgT = quartA.tile([P, KC * QW], f16, name=f"avgT_{q}", tag="avgT")
            catx8 = catx_cur
            catxv = catx8.rearrange("p (c t) -> p c t", c=KC)
            catf8 = quartA.tile([P, KC * QW], f8, name=f"catf_{q}", tag="catf")
            catfv = catf8.rearrange("p (c t) -> p c t", c=KC)
            r1T = quartB.tile([P, KC * QW], f16, name=f"r1T_{q}", tag="r1T")
            ffnT = quartB.tile([P, KC * QW], f16, name=f"ffnT_{q}", tag="ffnT")
            fnat = quartB.tile([P, QT * D], f16, name=f"fnat_{q}", tag="fnat")
            fv = fnat.rearrange("p (t d) -> p t d", t=QT)
            xi_tiles = []
            avgs = {}
            lns = {}
            avg_pend = [None]

            def emit_prefix(ti):
                """x load + triu cumsum + psA readout (avg scale) for tile ti.

                Emitted as early as possible so the ACT/DVE psA reads overlap
                whatever PE work precedes the strict-tril update."""
                i = q * QT + ti
                xi = xq_cur[ti]
                xi_tiles.append(xi)
                for s in range(2):
                    nc.tensor.matmul(psA[:, s * 512:(s + 1) * 512], triu,
                                     xi[:, s * 512:(s + 1) * 512],
                                     start=(i == 0), stop=False)
                avg_i = avgp.tile([P, D], f32r, name=f"avg_{i}", tag="avg")
                nc.scalar.mul(avg_i[:, 0:512], psA[:, 0:512], inv_sb[:, i:i + 1])
                nc.vector.tensor_scalar_mul(avg_i[:, 512:1024], psA[:, 512:1024],
                                            inv_sb[:, i:i + 1])
                return avg_i

            def flush_avgtr():
                """Deferred avg->avgT transposes of the previous tile; called
                at the top of arest so they fill the PE bubble while ACT/DVE
                read psA out (the role x-transposes played before catx moved
                to a host-side DMA)."""
                if avg_pend[0] is None:
                    return
                ti_, avg_, avgT_ = avg_pend[0]
                avg_pend[0] = None
                for g in range(2):
                    pt = ps512.tile([P, 512], f32, name=f"pta{q}_{ti_}_{g}",
                                    tag="tr")
                    for cc in range(4):
                        c = g * 4 + cc
                        nc.tensor.transpose(r(pt[:, cc * P:(cc + 1) * P]),
                                            avg_[:, c * P:(c + 1) * P], ident)
                    dst = wide3(avgT_, QW)[:, g * 4:(g + 1) * 4,
                                           ti_ * P:(ti_ + 1) * P]
                    nc.scalar.copy(dst, wide3(pt))

            def emit_arest(ti, avg_i):
                i = q * QT + ti
                xi = xi_tiles[ti]
                avgs[ti] = avg_i
                # PE bubble filler while ACT/DVE read psA out
                flush_avgtr()
                # psA += strict-lower-tril(x_i)  (now holds R_{i+1})
                for s in range(2):
                    nc.tensor.matmul(psA[:, s * 512:(s + 1) * 512], stril,
                                     xi[:, s * 512:(s + 1) * 512],
                                     start=False, stop=(i == NT - 1))

                # LN stats on avg_i, then per-tile rstd (fast-inverse-sqrt
                # seed + one Newton step on DVE -- no ACT sqrt table) and the
                # normalize, so each tile's chain hides in its own slack
                st6 = statp.tile([P, 12], f32, name=f"st6_{i}", tag="st6")
                nc.vector.bn_stats(st6[:, 0:6], v(avg_i[:, 0:512]))
                nc.vector.bn_stats(st6[:, 6:12], v(avg_i[:, 512:1024]))
                mv = statp.tile([P, 2], f32, name=f"mv_{i}", tag="mv")
                nc.vector.bn_aggr(mv, st6.rearrange("p (g s) -> p g s", g=2))
                avg_pend[0] = (ti, avg_i, avgT)

                ve = statp.tile([P, 1], f32, name=f"ve_{i}", tag="ve")
                nc.vector.tensor_scalar(ve, mv[:, 1:2], EPS, None, op0=Alu.add)
                sd = statp.tile([P, 1], f32, name=f"sd_{i}", tag="sd")
                nc.vector.tensor_scalar(sd.bitcast(i32), ve.bitcast(i32),
                                        1, 0xFFFFFFFF,
                                        op0=Alu.logical_shift_right,
                                        op1=Alu.bitwise_xor)
                nc.vector.tensor_scalar(sd.bitcast(i32), sd.bitcast(i32),
                                        0x5F3759E0, None, op0=Alu.add)
                hh = statp.tile([P, 1], f32, name=f"hh_{i}", tag="hh")
                nc.vector.tensor_tensor(hh, sd, sd, op=Alu.mult)
                nc.vector.tensor_tensor(hh, hh, ve, op=Alu.mult)
                nc.vector.tensor_scalar(hh, hh, -0.5, 1.5,
                                        op0=Alu.mult, op1=Alu.add)
                nc.vector.tensor_tensor(sd, sd, hh, op=Alu.mult)
                # ln = (avg - mean) * rstd into a separate buffer (avg_i
                # stays raw for the deferred avgT transposes)
                ln_i = avgp.tile([P, D], f32r, name=f"ln_{i}", tag="ln")
                nc.vector.tensor_scalar(ln_i, v(avg_i), mv[:, 0:1], sd,
                                        op0=Alu.subtract, op1=Alu.mult)
                lns[ti] = ln_i

            def emit_lntr_pair(t0, t1):
                """lnT transposes for a normalized tile pair (PE + ACT/DVE)."""
                for idx, ti_ in enumerate((t0, t1)):
                    avg_ = lns[ti_]
                    for g in range(2):
                        pt = ps512.tile([P, 512], f32, name=f"ptl{q}_{ti_}_{g}",
                                        tag="tr")
                        for cc in range(4):
                            c = g * 4 + cc
                            nc.tensor.transpose(r(pt[:, cc * P:(cc + 1) * P]),
                                                avg_[:, c * P:(c + 1) * P], ident)
                        dst = wide3(lnT, QW)[:, g * 4:(g + 1) * 4,
                                             ti_ * P:(ti_ + 1) * P]
                        if (idx + g) % 2 == 0:
                            nc.scalar.copy(dst, wide3(pt))
                        else:
                            nc.vector.tensor_copy(dst, wide3(pt))

            def emit_ffn_half(h2):
                """FFN on tokens [h2*256, h2*256+256) of this quarter."""
                c0 = h2 * 256
                if q == 0 and h2 == 0:
                    # k-OUTER y1 for the very first half: each w1 chunk is
                    # consumed as its DMA lands instead of stalling every
                    # n-block on the last chunk. Uses the gate PSUM banks
                    # (idle until ~40us) to keep 4 n-blocks live at once.
                    for g4 in range(2):
                        ps4 = gate_p.tile([P, D], f32, name=f"q0y1_{g4}",
                                          tag="g")
                        for k in range(KC):
                            for n4 in range(4):
                                n = g4 * 4 + n4
                                nc.tensor.matmul(
                                    ps4[:, n4 * 256:(n4 + 1) * 256],
                                    w1_sb[k][:, n * P:(n + 1) * P],
                                    lnT[:, k * QW + c0:k * QW + c0 + 256],
                                    start=(k == 0 and n4 % 2 == 0),
                                    stop=(k == KC - 1),
                                    skip_group_check=True)
                        for n4 in range(4):
                            n = g4 * 4 + n4
                            nc.scalar.activation(
                                r1T[:, n * QW + c0:n * QW + c0 + 256],
                                ps4[:, n4 * 256:(n4 + 1) * 256],
                                Act.Relu, bias=b1_sb[:, n:n + 1])
                else:
                    for n in range(KC):
                        ps = ps512.tile([P, 256], f32, name=f"ps1_{q}_{h2}_{n}",
                                        tag="tr")
                        for k in range(KC):
                            nc.tensor.matmul(
                                ps, w1_sb[k][:, n * P:(n + 1) * P],
                                lnT[:, k * QW + c0:k * QW + c0 + 256],
                                start=(k == 0), stop=(k == KC - 1))
                        nc.scalar.activation(
                            r1T[:, n * QW + c0:n * QW + c0 + 256],
                            ps, Act.Relu, bias=b1_sb[:, n:n + 1])
                for dch in range(KC):
                    ps = ps512.tile([P, 256], f32, name=f"ps2_{q}_{h2}_{dch}",
                                    tag="tr")
                    for k in range(KC):
                        nc.tensor.matmul(ps, w2_sb[k][:, dch * P:(dch + 1) * P],
                                         r1T[:, k * QW + c0:k * QW + c0 + 256],
                                         start=(k == 0), stop=(k == KC - 1))
                    # ffnT = (y2T + b2) + avgT  (f16 out)
                    sl = slice(dch * QW + c0, dch * QW + c0 + 256)
                    nc.vector.scalar_tensor_tensor(
                        ffnT[:, sl], ps, b2_sb[:, dch:dch + 1],
                        avgT[:, sl], op0=Alu.add, op1=Alu.add)
                    # fp8 shadow for the gating matmul (chunks 8..15); DVE --
                    # this feeds phase C's matmuls, and GPSIMD is ~3x slower
                    nc.vector.tensor_copy(catfv[:, dch, c0:c0 + 256],
                                          ffnT[:, sl])
                    # ffn back to natural layout, regrouped per token tile
                    pt = ps512.tile([P, 256], f16, name=f"ptf{q}_{h2}_{dch}",
                                    tag="tr")
                    for tt in range(2):
                        ti = 2 * h2 + tt
                        nc.tensor.transpose(
                            pt[:, tt * P:(tt + 1) * P],
                            ffnT[:, dch * QW + ti * P:dch * QW + (ti + 1) * P],
                            ident16)
                    dst = fv[:, 2 * h2:2 * h2 + 2, dch * P:(dch + 1) * P]
                    if dch % 2 == 0:
                        nc.scalar.copy(dst, wide3(pt))
                    else:
                        nc.vector.tensor_copy(dst, wide3(pt))
                for tt in range(2):
                    ti = 2 * h2 + tt
                    i = q * QT + ti
                    nc.sync.dma_start(out=ffn_d[i * P:(i + 1) * P, :],
                                      in_=fnat[:, ti * D:(ti + 1) * D])

            def emit_gate_tile(ti):
                """Phase C for one token tile: gating matmuls + epilogue."""
                i = q * QT + ti
                last = (i == NT - 1)
                prods = []
                for h in range(2):  # 0: input gate (j 0..1023), 1: forget gate
                    gps = gate_p.tile([P, D], f32, name=f"gps_{i}_{h}", tag="g")
                    for cp in range(NCP):
                        if cp < 4:
                            lhsT = catxv[:, 2 * cp:2 * cp + 2,
                                         ti * P:(ti + 1) * P]
                        elif cp < 8:
                            lhsT = catfv[:, 2 * (cp - 4):2 * (cp - 4) + 2,
                                         ti * P:(ti + 1) * P]
                        else:
                            lhsT = o2v
                        for jb in range(4):
                            j0 = h * D + jb * 256
                            # start only on the first write into each 2KB PSUM
                            # bank (start marks the whole bank pending-zero)
                            nc.tensor.matmul(gps[:, jb * 256:(jb + 1) * 256],
                                             lhsT,
                                             gwv[:, 2 * cp:2 * cp + 2, j0:j0 + 256],
                                             start=(cp == 0 and jb % 2 == 0),
                                             stop=(cp == NCP - 1),
                                             perf_mode=DR, skip_group_check=True)
                    sig = sigp.tile([P, D], f16, name=f"sig_{i}_{h}", tag="sig")
                    nc.scalar.activation(sig, gps, Act.Sigmoid)
                    # sig_ig*x on GPSIMD (idle engine), sig_fg*ffn on DVE;
                    # all-DVE on the last tile so the kernel tail is short
                    src = xi_tiles[ti] if h == 0 else fnat[:, ti * D:(ti + 1) * D]
                    prod = tmpp.tile([P, D], f16, name=f"prod_{i}_{h}",
                                     tag=f"prod{h}")
                    eng = nc.gpsimd if (h == 0 and not last) else nc.vector
                    eng.tensor_tensor(prod, sig, src, op=Alu.mult)
                    prods.append(prod)
                o = tmpp.tile([P, D], f16, name=f"o_{i}", tag="o")
                nc.vector.tensor_tensor(o, prods[0], prods[1], op=Alu.add)
                # in the drain-critical last quarter, spread stores over both
                # HW rings (ACT has no compute left after the final sigmoid)
                eng = nc.scalar if (q == NQ - 1 and ti % 2 == 1) else nc.sync
                eng.dma_start(out=out_d[i * P:(i + 1) * P, :], in_=o)

            # ---- phases A+B+C interleaved; stril(t2)/stril(t3) are deferred
            # ---- past B0 so B0's matmuls cover the psA readouts; C(t0,t1)
            # ---- run between the FFN halves so their epilogues overlap B1
            a0 = emit_prefix(0)
            emit_arest(0, a0)
            a1 = emit_prefix(1)
            emit_arest(1, a1)
            if q == 0:
                emit_w2_dmas()
            a2 = emit_prefix(2)
            emit_lntr_pair(0, 1)
            flush_avgtr()        # avgT(1), needed by ffn_half(0)'s stt
            emit_ffn_half(0)
            emit_arest(2, a2)
            a3 = emit_prefix(3)
            emit_arest(3, a3)
            if q == 0:
                emit_gw8_dmas()
                emit_catx_dmas(0, catx_cur)
            # prefetch next quarter's x tiles on sync ahead of the out stores
            if q + 1 < NQ:
                xq_next = []
                for ti in range(QT):
                    i2 = (q + 1) * QT + ti
                    xi = xload.tile([P, D], f16, name=f"xi_{i2}", tag="xi")
                    nc.sync.dma_start(out=xi, in_=x_d[i2 * P:(i2 + 1) * P, :])
                    xq_next.append(xi)
                xq_cur = xq_next
                catx_next = quartA.tile([P, KC * QW], f8,
                                        name=f"catx_{q + 1}", tag="catx")
                emit_catx_dmas(q + 1, catx_next)
                catx_cur = catx_next
            emit_lntr_pair(2, 3)
            flush_avgtr()        # avgT(3), needed by ffn_half(1)'s stt
            if q == 0:
                # gw8 (4MB) cannot land before ~30us no matter the queue
                # order; defer quarter 0's gating past B1 so the PE never
                # stalls on it
                emit_ffn_half(1)
                for ti in range(QT):
                    emit_gate_tile(ti)
            else:
                emit_gate_tile(0)
                emit_gate_tile(1)
                emit_ffn_half(1)
                emit_gate_tile(2)
                emit_gate_tile(3)

    nc.compile()
    _CACHE[key] = nc
    return nc


def _prep_maps(inputs, ln_g, ln_b, w1, b1, w2, b2, gw, gb):
    import ml_dtypes

    inputs = np.asarray(inputs, dtype=np.float32)
    ln_g = np.asarray(ln_g, dtype=np.float32)
    ln_b = np.asarray(ln_b, dtype=np.float32)
    w1 = np.asarray(w1, dtype=np.float32)
    b1 = np.asarray(b1, dtype=np.float32)
    w2 = np.asarray(w2, dtype=np.float32)
    b2 = np.asarray(b2, dtype=np.float32)
    gw = np.asarray(gw, dtype=np.float32)
    gb = np.asarray(gb, dtype=np.float32)

    has_bias = bool(np.any(gb != 0.0))
    GCH = 18 if has_bias else 16

    w1f = (ln_g[:, None] * w1).astype(np.float32)
    b1f = (ln_b @ w1 + b1).astype(np.float32)

    # gating weights: rows 0..2047 = gw (+ row 2048 = gb when nonzero); fp8.
    # layout [k within chunk, chunk c, j] so gwv[:, c, j] = gw_ext[c*128+k, j]
    gw_ext = np.zeros((GCH * P, 2 * D), np.float32)
    gw_ext[:2 * D] = gw
    if has_bias:
        gw_ext[2 * D] = gb
    gw8 = np.ascontiguousarray(
        gw_ext.reshape(GCH, P, 2 * D).transpose(1, 0, 2)
    ).astype(ml_dtypes.float8_e4m3).reshape(P, GCH * 2 * D)

    base = {
        "w1": np.ascontiguousarray(w1f.reshape(KC, P, D)).astype(np.float16),
        "b1": np.ascontiguousarray(b1f.reshape(KC, P).T),
        "w2": np.ascontiguousarray(w2.reshape(KC, P, D)).astype(np.float16),
        "b2": np.ascontiguousarray(b2.reshape(KC, P).T),
        "gw8": gw8,
        "invsteps": np.ascontiguousarray(
            (1.0 / np.arange(1, L + 1, dtype=np.float32)).reshape(NT, P).T),
        "triu": np.triu(np.ones((P, P), np.float16)),
        "stril": np.tril(np.ones((P, P), np.float16), -1),
        "ident": np.eye(P, dtype=np.float32),
        "ident16": np.eye(P, dtype=np.float16),
    }
    if has_bias:
        ones2 = np.zeros((P, 2 * P), np.float32)
        ones2[0, 0:P] = 1.0
        base["ones2"] = ones2.astype(ml_dtypes.float8_e4m3)
    maps = []
    for b in range(B):
        x16 = np.ascontiguousarray(inputs[b]).astype(np.float16)
        # host-transposed fp8 x for the gating matmul's stationary operand;
        # same f16 -> fp8e4 rounding the device cast used
        xt8 = np.ascontiguousarray(
            x16.astype(ml_dtypes.float8_e4m3)
               .reshape(NQ, QW, KC, P).transpose(0, 2, 3, 1))
        maps.append(dict(base, x=x16, xt8=xt8))
    return maps, has_bias


def _run(in_maps, has_bias, trace=False):
    from concourse.bass_utils import run_bass_kernel_spmd
    nc = _build(has_bias)
    return run_bass_kernel_spmd(nc, in_maps, list(range(B)), trace=trace)


def kernel(inputs, ln_g, ln_b, w1, b1, w2, b2, gw, gb):
    in_maps, has_bias = _prep_maps(inputs, ln_g, ln_b, w1, b1, w2, b2, gw, gb)
    res = _run(in_maps, has_bias).results
    out = np.stack([res[b]["out"].astype(np.float32) for b in range(B)])
    ffn = np.stack([res[b]["ffn"].astype(np.float32) for b in range(B)])
    return out, ffn


def kernel_traced(inputs, ln_g, ln_b, w1, b1, w2, b2, gw, gb):
    """Like kernel(), but also returns the BassKernelResults (with exec_time_ns)."""
    in_maps, has_bias = _prep_maps(inputs, ln_g, ln_b, w1, b1, w2, b2, gw, gb)
    bkr = _run(in_maps, has_bias, trace=True)
    res = bkr.results
    out = np.stack([res[b]["out"].astype(np.float32) for b in range(B)])
    ffn = np.stack([res[b]["ffn"].astype(np.float32) for b in range(B)])
    return (out, ffn), bkr
